# revision 1
# baseline (speedup 1.0000x reference)
"""Trainium2 Bass kernel for KeypointAlignmentLossL2.

Strategy (data-parallel over batch, one NeuronCore per batch element):
  Host prep (per core b):
    - transpose feat[b] from [C, H*W] to pixel-major [H*W, C], cast bf16
    - compute bilinear corner indices / weights from kp[b] (f32, exact
      floor/sub semantics; x0 clamped to W-2 with wx in [0,1] so all four
      corners are always in-bounds — identical math to the reference's
      zero-padded gather for coords in [0, W-1])
    - weights are packed as 128x128 bf16 diagonal matrices so the lerp can
      run on the tensor engine as accumulating diagonal matmuls
  Device (per core):
    - dma_gather: 4 corner rows (768 ch, bf16) per keypoint straight from
      HBM into SBUF, keypoint -> partition
    - TensorE: f = sum_nb diag(w_nb) @ g_nb accumulated in PSUM (f32)
    - ScalarE: copy f PSUM->SBUF
    - VectorE: fused tensor_tensor_reduce for ||f1||^2, ||f2||^2, <f1,f2>
    - outputs three [128, 8] f32 tiles (keypoint-chunk layout)
  Host finish: masked mean of 2 - 2*cos distances across all cores.
"""
import numpy as np
import ml_dtypes

B, C, H, W, N = 8, 768, 64, 64, 1024
HW_ = H * W
NCHUNK = N // 128  # 8 chunks of 128 keypoints
NQ = 4             # gather calls per image; each covers 2 chunks (1024 idxs)

_CACHE = {}


def _build_nc():
    from contextlib import ExitStack
    import concourse.bass as bass
    import concourse.tile as tile
    import concourse.mybir as mybir
    from concourse import bacc

    f32 = mybir.dt.float32
    bf16 = mybir.dt.bfloat16
    i16 = mybir.dt.int16

    nc = bacc.Bacc("TRN2", target_bir_lowering=False, debug=False, num_devices=8)

    featT1 = nc.dram_tensor("featT1", [HW_, C], bf16, kind="ExternalInput")
    featT2 = nc.dram_tensor("featT2", [HW_, C], bf16, kind="ExternalInput")
    idx1 = nc.dram_tensor("idx1", [128, 4 * N // 16], i16, kind="ExternalInput")
    idx2 = nc.dram_tensor("idx2", [128, 4 * N // 16], i16, kind="ExternalInput")
    wd = nc.dram_tensor("wd", [128, 2 * NCHUNK * 4, 128], bf16, kind="ExternalInput")
    out_n1 = nc.dram_tensor("out_n1", [128, NCHUNK], f32, kind="ExternalOutput")
    out_n2 = nc.dram_tensor("out_n2", [128, NCHUNK], f32, kind="ExternalOutput")
    out_dot = nc.dram_tensor("out_dot", [128, NCHUNK], f32, kind="ExternalOutput")

    featTs = (featT1, featT2)
    idxs_dram = (idx1, idx2)
    MULT = mybir.AluOpType.mult
    ADD = mybir.AluOpType.add

    with tile.TileContext(nc) as tc, ExitStack() as ctx:
        const_pool = ctx.enter_context(tc.tile_pool(name="const", bufs=1))
        gpool = ctx.enter_context(tc.tile_pool(name="g", bufs=4))
        fpool = ctx.enter_context(tc.tile_pool(name="f", bufs=4))
        dpool = ctx.enter_context(tc.tile_pool(name="d", bufs=2))
        ppool = ctx.enter_context(
            tc.tile_pool(name="p", bufs=8, space=bass.MemorySpace.PSUM)
        )

        wd_t = const_pool.tile([128, 2 * NCHUNK * 4, 128], bf16, tag="wd")
        nc.sync.dma_start(wd_t[:], wd[:])
        idx_t = []
        for im in range(2):
            t = const_pool.tile([128, 4 * N // 16], i16, tag=f"idx{im}", name=f"idx{im}")
            nc.sync.dma_start(t[:], idxs_dram[im][:])
            idx_t.append(t)

        res = []
        for name in ("n1", "n2", "dot"):
            res.append(const_pool.tile([128, NCHUNK], f32, tag=f"res_{name}", name=f"res_{name}"))

        for q in range(NQ):
            gt = []
            for im in range(2):
                g = gpool.tile([128, 2 * 4, C], bf16, tag="g")
                nc.gpsimd.dma_gather(
                    g[:],
                    featTs[im][:],
                    idx_t[im][:, q * 64:(q + 1) * 64],
                    1024,
                    1024,
                    C,
                )
                gt.append(g)
            for j in range(2):
                ch = 2 * q + j
                fs = []
                for im in range(2):
                    f_sb = fpool.tile([128, C], f32, tag="f")
                    for h in range(2):
                        ps = ppool.tile([128, C // 2], f32, tag="ps")
                        for nb in range(4):
                            nc.tensor.matmul(
                                ps[:],
                                wd_t[:, (im * NCHUNK + ch) * 4 + nb, :],
                                gt[im][:, 4 * j + nb, h * (C // 2):(h + 1) * (C // 2)],
                                start=(nb == 0),
                                stop=(nb == 3),
                            )
                        nc.scalar.copy(f_sb[:, h * (C // 2):(h + 1) * (C // 2)], ps[:])
                    fs.append(f_sb)
                dump_a = dpool.tile([128, C], f32, tag="dump_a", name="dump_a")
                dump_b = dpool.tile([128, C], f32, tag="dump_b", name="dump_b")
                nc.scalar.activation(
                    dump_a[:], fs[0][:], mybir.ActivationFunctionType.Square,
                    accum_out=res[0][:, ch:ch + 1],
                )
                nc.scalar.activation(
                    dump_a[:], fs[1][:], mybir.ActivationFunctionType.Square,
                    accum_out=res[1][:, ch:ch + 1],
                )
                nc.vector.tensor_tensor(dump_b[:], fs[0][:], fs[1][:], op=MULT)
                nc.vector.tensor_reduce(
                    res[2][:, ch:ch + 1], dump_b[:],
                    axis=mybir.AxisListType.X, op=ADD,
                )

        nc.sync.dma_start(out_n1[:], res[0][:])
        nc.sync.dma_start(out_n2[:], res[1][:])
        nc.sync.dma_start(out_dot[:], res[2][:])

    nc.compile()
    return nc


def get_nc():
    if "nc" not in _CACHE:
        _CACHE["nc"] = _build_nc()
    return _CACHE["nc"]


def _host_prep_img(feat_b, kp_b):
    """feat_b [C,H,W] f32, kp_b [N,2] f32 ->
    featT bf16 [HW_, C], nb_idx int32 [4, N], w f32 [4, N]"""
    featT = np.ascontiguousarray(
        np.asarray(feat_b, np.float32).reshape(C, HW_).T
    ).astype(ml_dtypes.bfloat16)
    x = np.asarray(kp_b[:, 0], np.float32)
    y = np.asarray(kp_b[:, 1], np.float32)
    x0 = np.minimum(np.floor(x), np.float32(W - 2)).astype(np.float32)
    y0 = np.minimum(np.floor(y), np.float32(H - 2)).astype(np.float32)
    wx = (x - x0).astype(np.float32)
    wy = (y - y0).astype(np.float32)
    pix = y0.astype(np.int32) * W + x0.astype(np.int32)
    nb_idx = np.stack([pix, pix + 1, pix + W, pix + W + 1], 0)
    w = np.stack(
        [(1 - wx) * (1 - wy), wx * (1 - wy), (1 - wx) * wy, wx * wy], 0
    ).astype(np.float32)
    return featT, nb_idx, w


def _make_idx_layout(nb_idx):
    """[4,N] corner indices -> [128, 4N/16] int16 SBUF index layout
    (element i=(4*ch+nb)*128+p lives at [i%16 (replicated x8), i//16])."""
    unwrapped = nb_idx.reshape(4, NCHUNK, 128).transpose(1, 0, 2).reshape(-1)
    lay = unwrapped.reshape(-1, 16).T
    return np.tile(lay, (8, 1)).astype(np.int16)


def _make_wd(w1, w2):
    """weights [4,N] f32 per image -> [128, 64, 128] bf16 diagonal matrices"""
    wd = np.zeros((128, 2 * NCHUNK * 4, 128), np.float32)
    r = np.arange(128)
    for im, w in ((0, w1), (1, w2)):
        for ch in range(NCHUNK):
            for nb in range(4):
                k = (im * NCHUNK + ch) * 4 + nb
                wd[r, k, r] = w[nb, ch * 128:(ch + 1) * 128]
    return wd.astype(ml_dtypes.bfloat16)


def kernel(feat1, feat2, kp1, kp2, kp1_mask, kp2_mask):
    from concourse.bass_utils import run_bass_kernel_spmd

    feat1 = np.asarray(feat1, np.float32)
    feat2 = np.asarray(feat2, np.float32)
    kp1 = np.asarray(kp1, np.float32)
    kp2 = np.asarray(kp2, np.float32)
    kp1_mask = np.asarray(kp1_mask)
    kp2_mask = np.asarray(kp2_mask)

    nc = get_nc()
    in_maps = []
    for b in range(B):
        fT1, nb1, w1 = _host_prep_img(feat1[b], kp1[b])
        fT2, nb2, w2 = _host_prep_img(feat2[b], kp2[b])
        in_maps.append({
            "featT1": fT1,
            "featT2": fT2,
            "idx1": _make_idx_layout(nb1),
            "idx2": _make_idx_layout(nb2),
            "wd": _make_wd(w1, w2),
        })

    results = run_bass_kernel_spmd(nc, in_maps, list(range(B))).results

    sum_l2 = 0.0
    sum_valid = 0.0
    for b in range(B):
        r = results[b]
        n1sq = r["out_n1"].T.reshape(-1).astype(np.float64)
        n2sq = r["out_n2"].T.reshape(-1).astype(np.float64)
        dot = r["out_dot"].T.reshape(-1).astype(np.float64)
        m1 = np.maximum(np.sqrt(n1sq), 1e-12)
        m2 = np.maximum(np.sqrt(n2sq), 1e-12)
        l2 = n1sq / (m1 * m1) + n2sq / (m2 * m2) - 2.0 * dot / (m1 * m2)
        valid = (kp1_mask[b] & kp2_mask[b]).astype(np.float64)
        sum_l2 += float((l2 * valid).sum())
        sum_valid += float(valid.sum())

    loss = 0.0 if sum_valid == 0 else sum_l2 / max(sum_valid, 1.0)
    return np.float32(loss)



# revision 6
# speedup vs baseline: 1.5202x; 1.5202x over previous
"""Trainium2 Bass kernel for KeypointAlignmentLossL2.

Strategy (data-parallel over batch, one NeuronCore per batch element):
  Host prep (per core b):
    - transpose feat[b] from [C, H*W] to pixel-major [H*W, C], cast fp8e4
    - bilinear corner indices / weights from kp[b] (x0 clamped to W-2 with
      wx in [0,1] -> exact bilinear semantics for coords in [0, W-1])
    - corners (y,x0),(y,x0+1) are ADJACENT rows in pixel-major layout, so
      each keypoint needs only TWO gather indices (elem_size=2 rows,
      elem_step=1 row), halving the Q7 descriptor-generation time that
      dominated the old kernel.
    - bilinear weights packed as 128x128 fp8 diagonal matrices (1 MB total)
  Device (per core):
    - dma_gather: 2 row-pair spans (2x768 ch, fp8) per keypoint from HBM
      into SBUF, keypoint -> partition
    - TensorE: f = sum_nb diag(w_nb) @ g_nb accumulated in PSUM (f32)
    - ScalarE: Square activation with accum_out -> ||f1||^2, ||f2||^2
      (reads PSUM directly, no copies)
    - VectorE: fused tensor_tensor_reduce -> <f1,f2> (reads PSUM directly)
  Host finish: masked mean of l2 distances across all cores.
"""
import numpy as np
import ml_dtypes

B, C, H, W, N = 8, 768, 64, 64, 1024
HW_ = H * W
NCHUNK = N // 128   # 8 chunks of 128 keypoints
NQ = 2              # gather calls per image; each covers 4 chunks (1024 idxs)
CH = C // 2         # 384, one PSUM bank of f32

F8 = ml_dtypes.float8_e4m3

_CACHE = {}


def _build_nc():
    from contextlib import ExitStack
    import concourse.bass as bass
    import concourse.tile as tile
    import concourse.mybir as mybir
    from concourse import bacc

    f32 = mybir.dt.float32
    f8 = mybir.dt.float8e4
    i16 = mybir.dt.int16

    nc = bacc.Bacc("TRN2", target_bir_lowering=False, debug=False, num_devices=8)

    featT1 = nc.dram_tensor("featT1", [HW_, C], f8, kind="ExternalInput")
    featT2 = nc.dram_tensor("featT2", [HW_, C], f8, kind="ExternalInput")
    # 2048 pair-indices per image, wrapped [16, 128] and replicated to 128 rows
    idx1 = nc.dram_tensor("idx1", [128, 2 * N // 16], i16, kind="ExternalInput")
    idx2 = nc.dram_tensor("idx2", [128, 2 * N // 16], i16, kind="ExternalInput")
    # diag weight mats: k = (im*NCHUNK + c)*4 + nb
    wd = nc.dram_tensor("wd", [128, 2 * NCHUNK * 4, 128], f8, kind="ExternalInput")
    out_n1 = nc.dram_tensor("out_n1", [128, 2 * NCHUNK], f32, kind="ExternalOutput")
    out_n2 = nc.dram_tensor("out_n2", [128, 2 * NCHUNK], f32, kind="ExternalOutput")
    out_dot = nc.dram_tensor("out_dot", [128, 2 * NCHUNK], f32, kind="ExternalOutput")

    idxs_dram = (idx1, idx2)
    MULT = mybir.AluOpType.mult
    ADD = mybir.AluOpType.add

    def pair_ap(featd):
        # overlapping view [HW-1, 2C] stride (C, 1): idx granularity one
        # 768-ch row, each gather grabs two adjacent rows
        ap = featd[:]
        ap.ap[0] = [C, HW_ - 1]
        ap.ap[1] = [1, 2 * C]
        return ap

    feat_aps = (pair_ap(featT1), pair_ap(featT2))

    with tile.TileContext(nc) as tc, ExitStack() as ctx:
        const_pool = ctx.enter_context(tc.tile_pool(name="const", bufs=1))
        gpool = ctx.enter_context(tc.tile_pool(name="g", bufs=2))
        dpool = ctx.enter_context(tc.tile_pool(name="d", bufs=2))
        ppool = ctx.enter_context(
            tc.tile_pool(name="p", bufs=2, space=bass.MemorySpace.PSUM)
        )

        idx_t = []
        for im in range(2):
            t = const_pool.tile([128, 2 * N // 16], i16, tag=f"idx{im}", name=f"idx{im}")
            nc.sync.dma_start(t[:], idxs_dram[im][:])
            idx_t.append(t)
        wd_t = const_pool.tile([128, 2 * NCHUNK * 4, 128], f8, tag="wd")
        nc.sync.dma_start(wd_t[:], wd[:])

        res = []
        for name in ("n1", "n2", "dot"):
            res.append(const_pool.tile([128, 2 * NCHUNK], f32,
                                       tag=f"res_{name}", name=f"res_{name}"))

        for q in range(NQ):
            gt = []
            for im in range(2):
                g = gpool.tile([128, 2 * 4, 2 * C], f8, tag=f"g{im}")
                nc.gpsimd.dma_gather(
                    g[:],
                    feat_aps[im],
                    idx_t[im][:, q * 64:(q + 1) * 64],
                    1024,
                    1024,
                    2 * C,
                    elem_step=C,
                )
                gt.append(g)
            for c_loc in range(4):
                c = 4 * q + c_loc
                pss = []
                for im in range(2):
                    ps = [ppool.tile([128, CH], f32, tag=f"ps{im}{h}",
                                     name=f"ps{im}{h}") for h in range(2)]
                    for nb in range(4):
                        j, x = nb // 2, nb % 2
                        lhsT = wd_t[:, (im * NCHUNK + c) * 4 + nb, :]
                        for h in range(2):
                            rhs = gt[im][:, 2 * c_loc + j,
                                         x * C + h * CH: x * C + (h + 1) * CH]
                            nc.tensor.matmul(
                                ps[h][:], lhsT, rhs,
                                start=(nb == 0), stop=(nb == 3),
                            )
                    pss.append(ps)
                for h in range(2):
                    col = 2 * c + h
                    da = dpool.tile([128, CH], f32, tag=f"da{h}", name=f"da{h}")
                    db = dpool.tile([128, CH], f32, tag=f"db{h}", name=f"db{h}")
                    dd = dpool.tile([128, CH], f32, tag=f"dd{h}", name=f"dd{h}")
                    f1s = dpool.tile([128, CH], f32, tag=f"f1s{h}", name=f"f1s{h}")
                    nc.scalar.activation(
                        da[:], pss[0][h][:], mybir.ActivationFunctionType.Square,
                        accum_out=res[0][:, col:col + 1],
                    )
                    nc.scalar.activation(
                        db[:], pss[1][h][:], mybir.ActivationFunctionType.Square,
                        accum_out=res[1][:, col:col + 1],
                    )
                    # DVE may read only one PSUM operand: stage f1 in SBUF.
                    # (tensor_tensor_reduce mis-executes on HW; use the
                    # scalar_tensor_tensor accum_out fusion instead.)
                    nc.vector.tensor_copy(f1s[:], pss[0][h][:])
                    nc.vector.scalar_tensor_tensor(
                        dd[:], pss[1][h][:], 1.0, f1s[:],
                        MULT, MULT, accum_out=res[2][:, col:col + 1],
                    )

        nc.sync.dma_start(out_n1[:], res[0][:])
        nc.sync.dma_start(out_n2[:], res[1][:])
        nc.sync.dma_start(out_dot[:], res[2][:])

    nc.compile()
    return nc


def get_nc():
    if "nc" not in _CACHE:
        _CACHE["nc"] = _build_nc()
    return _CACHE["nc"]


def _host_prep_img(feat_b, kp_b):
    """feat_b [C,H,W] f32, kp_b [N,2] f32 ->
    featT fp8 [HW_, C], pix0 int32 [N], w f32 [4, N]"""
    featT = np.ascontiguousarray(
        np.asarray(feat_b, np.float32).reshape(C, HW_).T
    )
    featT = np.clip(featT, -240.0, 240.0).astype(F8)
    x = np.asarray(kp_b[:, 0], np.float32)
    y = np.asarray(kp_b[:, 1], np.float32)
    x0 = np.minimum(np.floor(x), np.float32(W - 2)).astype(np.float32)
    y0 = np.minimum(np.floor(y), np.float32(H - 2)).astype(np.float32)
    wx = (x - x0).astype(np.float32)
    wy = (y - y0).astype(np.float32)
    pix0 = y0.astype(np.int32) * W + x0.astype(np.int32)
    w = np.stack(
        [(1 - wx) * (1 - wy), wx * (1 - wy), (1 - wx) * wy, wx * wy], 0
    ).astype(np.float32)
    return featT, pix0, w


def _make_idx_layout(pix0):
    """pix0 [N] -> [128, 2N/16] int16 SBUF index layout.
    Gather position i = q*1024 + s*128 + p (s in [0,8)): chunk c = 4q + s//2,
    y-half j = s%2, keypoint k = c*128 + p, index value = pix0[k] + 64*j."""
    vals = np.empty(2 * N, np.int32)
    i = np.arange(2 * N)
    q, r = i // 1024, i % 1024
    s, p = r // 128, r % 128
    cc, j = 4 * q + s // 2, s % 2
    vals = pix0[cc * 128 + p] + W * j
    lay = vals.reshape(-1, 16).T
    return np.tile(lay, (8, 1)).astype(np.int16)


def _make_wd(w1, w2):
    """weights [4,N] f32 per image -> [128, 64, 128] fp8 diagonal matrices"""
    wd = np.zeros((128, 2 * NCHUNK * 4, 128), np.float32)
    r = np.arange(128)
    for im, w in ((0, w1), (1, w2)):
        for ch in range(NCHUNK):
            for nb in range(4):
                k = (im * NCHUNK + ch) * 4 + nb
                wd[r, k, r] = w[nb, ch * 128:(ch + 1) * 128]
    return wd.astype(F8)


def _host_inputs(feat1, feat2, kp1, kp2):
    in_maps = []
    for b in range(B):
        fT1, pix1, w1 = _host_prep_img(feat1[b], kp1[b])
        fT2, pix2, w2 = _host_prep_img(feat2[b], kp2[b])
        in_maps.append({
            "featT1": fT1,
            "featT2": fT2,
            "idx1": _make_idx_layout(pix1),
            "idx2": _make_idx_layout(pix2),
            "wd": _make_wd(w1, w2),
        })
    return in_maps


def kernel(feat1, feat2, kp1, kp2, kp1_mask, kp2_mask):
    from concourse.bass_utils import run_bass_kernel_spmd

    feat1 = np.asarray(feat1, np.float32)
    feat2 = np.asarray(feat2, np.float32)
    kp1 = np.asarray(kp1, np.float32)
    kp2 = np.asarray(kp2, np.float32)
    kp1_mask = np.asarray(kp1_mask)
    kp2_mask = np.asarray(kp2_mask)

    nc = get_nc()
    in_maps = _host_inputs(feat1, feat2, kp1, kp2)
    results = run_bass_kernel_spmd(nc, in_maps, list(range(B))).results

    sum_l2 = 0.0
    sum_valid = 0.0
    for b in range(B):
        r = results[b]
        # col = 2*c + h -> sum halves, then flatten chunk-major
        def unpack(a):
            return (a.reshape(128, NCHUNK, 2).sum(-1)
                    .T.reshape(-1).astype(np.float64))
        n1sq = unpack(r["out_n1"])
        n2sq = unpack(r["out_n2"])
        dot = unpack(r["out_dot"])
        m1 = np.maximum(np.sqrt(n1sq), 1e-12)
        m2 = np.maximum(np.sqrt(n2sq), 1e-12)
        l2 = n1sq / (m1 * m1) + n2sq / (m2 * m2) - 2.0 * dot / (m1 * m2)
        valid = (kp1_mask[b] & kp2_mask[b]).astype(np.float64)
        sum_l2 += float((l2 * valid).sum())
        sum_valid += float(valid.sum())

    loss = 0.0 if sum_valid == 0 else sum_l2 / max(sum_valid, 1.0)
    return np.float32(loss)


# revision 7
# speedup vs baseline: 1.6467x; 1.0832x over previous
"""Trainium2 Bass kernel for KeypointAlignmentLossL2 — split-path version.

Data-parallel over batch (1 NeuronCore per batch element). The dot product
needs f1[kp] and f2[kp] on the same partition, so both images share one
keypoint ordering: kps are sorted by their img2 pixel-row index.

  - img2: full pixel-major feature map bulk-loaded to SBUF (fp8, 3.1 MB,
    HWDGE — zero Q7 time). Sorted chunks of 128 kps touch only a narrow
    window of 128-pixel-row tiles, so gather+bilinear-lerp fuses into a few
    accumulating one-hot matmuls per chunk (sparse weight mats built on
    host, ~0.9 MB).
  - img1: dma_gather of 2-row pair spans per kp (sorted order; gather does
    not care about locality). Only image 1 pays Q7 descriptor-gen time
    (~17 us). A dummy 16-idx gather issued first forces the ~6 us Q7
    gather-ucode IRAM load to overlap the input DMAs.
  - lerp img1: diagonal-matrix matmuls; squares on ScalarE (PSUM read,
    accum_out); dot via scalar_tensor_tensor accum_out fusion on VectorE
    (tensor_tensor_reduce mis-executes on HW).
  Host finishes: masked mean of l2 distances across cores.
"""
import numpy as np
import ml_dtypes

B, C, H, W, N = 8, 768, 64, 64, 1024
HW_ = H * W
NCHUNK = N // 128   # 8 chunks of 128 keypoints
NTILE = HW_ // 128  # 32 map tiles of 128 pixel-rows
CH = C // 2         # 384 = one PSUM bank of f32
NG = 4              # img1 gather calls (512 idxs = 2 chunks each)

F8 = ml_dtypes.float8_e4m3

_CACHE = {}


def _build_nc(windows):
    """windows: tuple of (t_lo, span) per chunk for the img2 one-hot path."""
    from contextlib import ExitStack
    import concourse.bass as bass
    import concourse.tile as tile
    import concourse.mybir as mybir
    from concourse import bacc

    f32 = mybir.dt.float32
    f8 = mybir.dt.float8e4
    i16 = mybir.dt.int16

    sums = sum(s for _, s in windows)
    offs = np.cumsum([0] + [s for _, s in windows])[:-1]

    nc = bacc.Bacc("TRN2", target_bir_lowering=False, debug=False, num_devices=8)

    featT1 = nc.dram_tensor("featT1", [HW_, C], f8, kind="ExternalInput")
    featI2 = nc.dram_tensor("featI2", [128, NTILE * C], f8, kind="ExternalInput")
    idx1 = nc.dram_tensor("idx1", [128, 2 * N // 16], i16, kind="ExternalInput")
    wd1 = nc.dram_tensor("wd1", [128, NCHUNK * 4, 128], f8, kind="ExternalInput")
    w2m = nc.dram_tensor("w2m", [128, sums, 128], f8, kind="ExternalInput")
    out_n1 = nc.dram_tensor("out_n1", [128, 2 * NCHUNK], f32, kind="ExternalOutput")
    out_n2 = nc.dram_tensor("out_n2", [128, 2 * NCHUNK], f32, kind="ExternalOutput")
    out_dot = nc.dram_tensor("out_dot", [128, 2 * NCHUNK], f32, kind="ExternalOutput")

    MULT = mybir.AluOpType.mult

    fap1 = featT1[:]
    fap1.ap[0] = [C, HW_ - 1]
    fap1.ap[1] = [1, 2 * C]

    with tile.TileContext(nc) as tc, ExitStack() as ctx:
        const_pool = ctx.enter_context(tc.tile_pool(name="const", bufs=1))
        gpool = ctx.enter_context(tc.tile_pool(name="g", bufs=3))
        dpool = ctx.enter_context(tc.tile_pool(name="d", bufs=2))
        ppool = ctx.enter_context(
            tc.tile_pool(name="p", bufs=2, space=bass.MemorySpace.PSUM)
        )

        idx_t = const_pool.tile([128, 2 * N // 16], i16, tag="idx1", name="idx1")
        nc.sync.dma_start(idx_t[:], idx1[:])

        # dummy gather: forces the Q7 gather-ucode IRAM load (~6 us) to
        # happen while the input DMAs are still in flight
        idxd = const_pool.tile([128, 1], i16, tag="idxd", name="idxd")
        nc.vector.memset(idxd[:], 0)
        gd = const_pool.tile([128, 1, 2 * C], f8, tag="gd", name="gd")
        nc.gpsimd.dma_gather(gd[:], fap1, idxd[:], 16, 16, 2 * C, elem_step=C)

        map_t = []
        for a in range(4):
            t = const_pool.tile([128, 8, C], f8, tag=f"map{a}", name=f"map{a}")
            nc.sync.dma_start(t[:], featI2[:, a * 8 * C:(a + 1) * 8 * C])
            map_t.append(t)
        w2_t = const_pool.tile([128, sums, 128], f8, tag="w2m", name="w2m")
        nc.sync.dma_start(w2_t[:], w2m[:])
        wd_t = const_pool.tile([128, NCHUNK * 4, 128], f8, tag="wd1", name="wd1")
        nc.sync.dma_start(wd_t[:], wd1[:])

        res = []
        for nm in ("n1", "n2", "dot"):
            res.append(const_pool.tile([128, 2 * NCHUNK], f32,
                                       tag=f"res_{nm}", name=f"res_{nm}"))

        gt = []
        for q in range(NG):
            g = gpool.tile([128, 4, 2 * C], f8, tag="g1", name="g1")
            nc.gpsimd.dma_gather(
                g[:], fap1, idx_t[:, q * 32:(q + 1) * 32],
                512, 512, 2 * C, elem_step=C,
            )
            gt.append(g)

        for cch in range(NCHUNK):
            q, c_loc = cch // 2, cch % 2
            t_lo, span = windows[cch]
            # img2: one-hot gather+lerp matmuls over map window
            ps2 = [ppool.tile([128, CH], f32, tag=f"ps2{h}", name=f"ps2{h}")
                   for h in range(2)]
            for i in range(span):
                t = t_lo + i
                lhsT = w2_t[:, int(offs[cch]) + i, :]
                for h in range(2):
                    rhs = map_t[t // 8][:, t % 8, h * CH:(h + 1) * CH]
                    nc.tensor.matmul(
                        ps2[h][:], lhsT, rhs,
                        start=(i == 0), stop=(i == span - 1),
                    )
            # img1: diagonal lerp matmuls over gathered pairs
            ps1 = [ppool.tile([128, CH], f32, tag=f"ps1{h}", name=f"ps1{h}")
                   for h in range(2)]
            for nb in range(4):
                j, x = nb // 2, nb % 2
                lhsT = wd_t[:, cch * 4 + nb, :]
                for h in range(2):
                    rhs = gt[q][:, 2 * c_loc + j,
                                x * C + h * CH: x * C + (h + 1) * CH]
                    nc.tensor.matmul(
                        ps1[h][:], lhsT, rhs,
                        start=(nb == 0), stop=(nb == 3),
                    )
            for h in range(2):
                col = 2 * cch + h
                da = dpool.tile([128, CH], f32, tag=f"da{h}", name=f"da{h}")
                db = dpool.tile([128, CH], f32, tag=f"db{h}", name=f"db{h}")
                dd = dpool.tile([128, CH], f32, tag=f"dd{h}", name=f"dd{h}")
                f1s = dpool.tile([128, CH], f32, tag=f"f1s{h}", name=f"f1s{h}")
                nc.scalar.activation(
                    da[:], ps1[h][:], mybir.ActivationFunctionType.Square,
                    accum_out=res[0][:, col:col + 1],
                )
                nc.scalar.activation(
                    db[:], ps2[h][:], mybir.ActivationFunctionType.Square,
                    accum_out=res[1][:, col:col + 1],
                )
                # DVE may read only one PSUM operand: stage f1 in SBUF
                nc.vector.tensor_copy(f1s[:], ps1[h][:])
                nc.vector.scalar_tensor_tensor(
                    dd[:], ps2[h][:], 1.0, f1s[:],
                    MULT, MULT, accum_out=res[2][:, col:col + 1],
                )

        nc.sync.dma_start(out_n1[:], res[0][:])
        nc.sync.dma_start(out_n2[:], res[1][:])
        nc.sync.dma_start(out_dot[:], res[2][:])

    nc.compile()
    return nc


def get_nc(windows):
    key = tuple(windows)
    if key not in _CACHE:
        _CACHE[key] = _build_nc(windows)
    return _CACHE[key]


def _corner_data(kp_b):
    x = np.asarray(kp_b[:, 0], np.float32)
    y = np.asarray(kp_b[:, 1], np.float32)
    x0 = np.minimum(np.floor(x), np.float32(W - 2)).astype(np.float32)
    y0 = np.minimum(np.floor(y), np.float32(H - 2)).astype(np.float32)
    wx = (x - x0).astype(np.float32)
    wy = (y - y0).astype(np.float32)
    pix0 = y0.astype(np.int32) * W + x0.astype(np.int32)
    w = np.stack(
        [(1 - wx) * (1 - wy), wx * (1 - wy), (1 - wx) * wy, wx * wy], 0
    ).astype(np.float32)
    return pix0, w


def _featT_f8(feat_b):
    featT = np.ascontiguousarray(np.asarray(feat_b, np.float32).reshape(C, HW_).T)
    return np.clip(featT, -240.0, 240.0).astype(F8)


def _make_idx_layout(pix0s):
    """sorted pair-start rows [N] -> [128, 2N/16] int16 gather-index layout.
    Call q (512 idxs) covers chunks [2q, 2q+2): position i = q*512+s*128+p,
    chunk c = 2q + s//2, y-half j = s%2, value = pix0s[c*128+p] + 64*j."""
    i = np.arange(2 * N)
    q, r = i // 512, i % 512
    s, p = r // 128, r % 128
    cc, j = 2 * q + s // 2, s % 2
    vals = pix0s[cc * 128 + p] + W * j
    lay = vals.reshape(-1, 16).T
    return np.tile(lay, (8, 1)).astype(np.int16)


def _make_wd1(w1s):
    wd = np.zeros((128, NCHUNK * 4, 128), np.float32)
    r = np.arange(128)
    for ch in range(NCHUNK):
        for nb in range(4):
            wd[r, ch * 4 + nb, r] = w1s[nb, ch * 128:(ch + 1) * 128]
    return wd.astype(F8)


def _chunk_ranges(pix2s):
    """per-chunk (t_lo, t_hi) of map tiles touched by img2 corners"""
    out = []
    for cc in range(NCHUNK):
        pp = pix2s[cc * 128:(cc + 1) * 128]
        t_lo = int(pp.min()) // 128
        t_hi = (int(pp.max()) + W + 1) // 128
        out.append((t_lo, t_hi))
    return out


def _make_w2m(pix2s, w2s, windows):
    sums = sum(s for _, s in windows)
    w2m = np.zeros((128, sums, 128), np.float32)
    off = 0
    for cc in range(NCHUNK):
        t_lo, span = windows[cc]
        k = np.arange(128)
        for nb in range(4):
            pix = pix2s[cc * 128:(cc + 1) * 128] + (nb % 2) + W * (nb // 2)
            blk = pix // 128 - t_lo
            row = pix % 128
            np.add.at(w2m, (row, off + blk, k), w2s[nb, cc * 128:(cc + 1) * 128])
        off += span
    return w2m.astype(F8)


def _host_inputs(feat1, feat2, kp1, kp2):
    """returns (in_maps, perms, windows)"""
    pre = []
    ranges = []
    for b in range(B):
        pix1, w1 = _corner_data(kp1[b])
        pix2, w2 = _corner_data(kp2[b])
        perm = np.argsort(pix2, kind="stable")
        pre.append((pix1[perm], w1[:, perm], pix2[perm], w2[:, perm], perm))
        ranges.append(_chunk_ranges(pre[-1][2]))
    # shared windows across cores (SPMD: one program for all 8)
    windows = []
    for cc in range(NCHUNK):
        t_lo = min(r[cc][0] for r in ranges)
        t_hi = max(r[cc][1] for r in ranges)
        windows.append((t_lo, t_hi - t_lo + 1))
    windows = tuple(windows)

    in_maps = []
    perms = []
    for b in range(B):
        pix1s, w1s, pix2s, w2s, perm = pre[b]
        in_maps.append({
            "featT1": _featT_f8(feat1[b]),
            "featI2": np.ascontiguousarray(
                _featT_f8(feat2[b]).reshape(NTILE, 128, C)
                .transpose(1, 0, 2).reshape(128, NTILE * C)),
            "idx1": _make_idx_layout(pix1s),
            "wd1": _make_wd1(w1s),
            "w2m": _make_w2m(pix2s, w2s, windows),
        })
        perms.append(perm)
    return in_maps, perms, windows


def kernel(feat1, feat2, kp1, kp2, kp1_mask, kp2_mask):
    from concourse.bass_utils import run_bass_kernel_spmd

    feat1 = np.asarray(feat1, np.float32)
    feat2 = np.asarray(feat2, np.float32)
    kp1 = np.asarray(kp1, np.float32)
    kp2 = np.asarray(kp2, np.float32)
    kp1_mask = np.asarray(kp1_mask)
    kp2_mask = np.asarray(kp2_mask)

    in_maps, perms, windows = _host_inputs(feat1, feat2, kp1, kp2)
    nc = get_nc(windows)
    results = run_bass_kernel_spmd(nc, in_maps, list(range(B))).results

    sum_l2 = 0.0
    sum_valid = 0.0
    for b in range(B):
        r = results[b]

        def unpack(a):
            return (a.reshape(128, NCHUNK, 2).sum(-1)
                    .T.reshape(-1).astype(np.float64))
        n1sq = unpack(r["out_n1"])
        n2sq = unpack(r["out_n2"])
        dot = unpack(r["out_dot"])
        m1 = np.maximum(np.sqrt(n1sq), 1e-12)
        m2 = np.maximum(np.sqrt(n2sq), 1e-12)
        l2 = n1sq / (m1 * m1) + n2sq / (m2 * m2) - 2.0 * dot / (m1 * m2)
        valid = (kp1_mask[b] & kp2_mask[b]).astype(np.float64)[perms[b]]
        sum_l2 += float((l2 * valid).sum())
        sum_valid += float(valid.sum())

    loss = 0.0 if sum_valid == 0 else sum_l2 / max(sum_valid, 1.0)
    return np.float32(loss)
